# revision 1
# baseline (speedup 1.0000x reference)
"""Trainium2 Bass kernel for nn_EuclideanExperts (8-expert 2-layer GraphSAGE).

Expert-parallel: each of the 8 NeuronCores runs one expert's full encoder.
The graph aggregation (mean over in-neighbors) is computed as a sequence of
one-hot matmuls: edges sorted by destination window are gathered 128 at a
time with dma_gather (bf16 rows), a one-hot selection matrix S is built on
the vector engine from destination offsets, and S.T @ G accumulates into a
PSUM tile holding the window's aggregate.  Dense layer matmuls, BatchNorm
and ReLU run in a feature-major layout (features on partitions) so BN
reductions are free-axis reductions and the BN+ReLU apply is a single
scalar-engine activation per window.

Self-contained: only numpy + the concourse stack from /opt/trn_rl_repo.
"""
import sys

for _p in ("/opt/trn_rl_repo", "/root/.axon_site/_ro/trn_rl_repo"):
    if _p not in sys.path:
        sys.path.insert(0, _p)

import os

import numpy as np
import ml_dtypes

import concourse.bacc as bacc
import concourse.mybir as mybir
import concourse.tile as tile
from concourse.bass_utils import run_bass_kernel_spmd

F32 = mybir.dt.float32
BF16 = mybir.dt.bfloat16
I16 = mybir.dt.int16
AX = mybir.AxisListType
OP = mybir.AluOpType
AF = mybir.ActivationFunctionType

EPS = 1e-5


# --------------------------------------------------------------------------
# host-side graph preprocessing (index data only; no float math on x)
# --------------------------------------------------------------------------
def preprocess(edge_index, n_nodes, block=8, group=25000, gmax=4096):
    """Sort edges into (window-block, src-group) runs and chunk them.

    Returns metadata driving the bass program plus the packed index arrays.
    A "window" is 128 consecutive destination nodes (one PSUM tile's rows).
    A "run" is all edges with dst in one block and src in one group, sorted
    by dst, split into gathers of <= gmax indices (multiple of 128).
    """
    src = np.asarray(edge_index[0], dtype=np.int64)
    dst = np.asarray(edge_index[1], dtype=np.int64)
    E = src.shape[0]
    nw = (n_nodes + 127) // 128
    n_groups = (n_nodes + group - 1) // group

    deg = np.bincount(dst, minlength=n_nodes).astype(np.float32)
    inv_deg = (1.0 / np.maximum(deg, 1.0)).astype(np.float32)
    inv_pad = np.ones(nw * 128, np.float32)
    inv_pad[:n_nodes] = inv_deg
    inv_col = inv_pad.reshape(nw, 128).T.copy()  # [128, nw]

    # order all edges by (block, group, dst) in a single argsort
    blk = dst // (128 * block)
    grp = src // group
    key = (blk * n_groups + grp) * np.int64(n_nodes) + dst
    order = np.argsort(key, kind="stable")
    s_s, s_d, s_b, s_g = src[order], dst[order], blk[order], grp[order]

    # run boundaries: change of (blk, grp)
    rk = s_b * n_groups + s_g
    bounds = np.flatnonzero(np.diff(rk)) + 1
    starts = np.concatenate([[0], bounds])
    ends = np.concatenate([bounds, [E]])

    gathers = []          # per gather: dict(g, idx_off, nidx, dl_off, chunks)
    idx_parts = []        # wrapped int16 [16, n/16] pieces (replicated later)
    dl_parts = []         # [128, C] f32 pieces
    idx_cursor = 0        # in int16 columns (16-wrapped)
    dl_cursor = 0         # in chunks
    for s, e in zip(starts, ends):
        g = int(s_g[s])
        for q in range(s, e, gmax):
            qe = min(q + gmax, e)
            es = s_s[q:qe] - g * group
            ed = s_d[q:qe]
            ne = qe - q
            npad = (-ne) % 128
            idxs = np.concatenate([es, np.zeros(npad, np.int64)])
            dabs = np.concatenate([ed, np.full(npad, -1, np.int64)])
            nidx = ne + npad
            C = nidx // 128
            chunks = []
            dl_rel = np.empty(nidx, np.float32)
            for c in range(C):
                dc = dabs[c * 128:(c + 1) * 128]
                valid = dc >= 0
                wfirst = int(dc[valid].min()) // 128
                wlast = int(dc[valid].max()) // 128
                dl_rel[c * 128:(c + 1) * 128] = np.where(
                    valid, dc - wfirst * 128, -1)
                chunks.append((wfirst, wlast - wfirst + 1))
            idx_parts.append(idxs.reshape(-1, 16).T.astype(np.int16))
            dl_parts.append(dl_rel.reshape(C, 128).T)
            gathers.append(dict(g=g, idx_off=idx_cursor, nidx=nidx,
                                dl_off=dl_cursor, chunks=chunks))
            idx_cursor += nidx // 16
            dl_cursor += C

    idx_arr = np.tile(np.concatenate(idx_parts, axis=1), (8, 1))  # [128, TI]
    dl_arr = np.concatenate(dl_parts, axis=1).astype(np.float32)

    # per-window first/last matmul id for start/stop flags
    first_mm = {}
    last_mm = {}
    for gi, ga in enumerate(gathers):
        for ci, (wf, span) in enumerate(ga["chunks"]):
            for k in range(span):
                w = wf + k
                first_mm.setdefault(w, (gi, ci, k))
                last_mm[w] = (gi, ci, k)

    return dict(gathers=gathers, idx_arr=idx_arr, dl_arr=dl_arr,
                inv_col=inv_col, first_mm=first_mm, last_mm=last_mm,
                nw=nw, n_groups=n_groups, block=block, group=group)


# --------------------------------------------------------------------------
# bass program
# --------------------------------------------------------------------------
def build_program(meta, n_nodes, d=128):
    nw = meta["nw"]
    block = meta["block"]
    group = meta["group"]
    gathers = meta["gathers"]
    first_mm = meta["first_mm"]
    last_mm = meta["last_mm"]
    TI = meta["idx_arr"].shape[1]
    TC = meta["dl_arr"].shape[1]

    def wsz(w):
        return min(128, n_nodes - w * 128)

    n_queues = int(os.environ.get("KERNEL_QUEUES", "1"))
    nc = bacc.Bacc("TRN2", target_bir_lowering=False, debug=False,
                   num_swdge_queues=n_queues)
    nc._kq = n_queues
    x_t = nc.declare_dram_parameter("x", [n_nodes, d], F32, isOutput=False)
    idx_t = nc.declare_dram_parameter("idx", [128, TI], I16, isOutput=False)
    dl_t = nc.declare_dram_parameter("dl", [128, TC], F32, isOutput=False)
    invd_t = nc.declare_dram_parameter("invd", [128, nw], F32, isOutput=False)
    iota_t = nc.declare_dram_parameter("iota", [128, 128], F32, isOutput=False)
    ident_t = nc.declare_dram_parameter("ident", [128, 128], F32, isOutput=False)
    identb_t = nc.declare_dram_parameter("identb", [128, 128], BF16, isOutput=False)
    ws_t = nc.declare_dram_parameter("Wself", [2, d, d], F32, isOutput=False)
    wn_t = nc.declare_dram_parameter("Wnbr", [2, d, d], F32, isOutput=False)
    wsb_t = nc.declare_dram_parameter("Wselfb", [d, d], BF16, isOutput=False)  # layer-1 self, bf16
    b_t = nc.declare_dram_parameter("bias", [2, d, 1], F32, isOutput=False)
    gam_t = nc.declare_dram_parameter("gamma", [d, 1], F32, isOutput=False)
    bet_t = nc.declare_dram_parameter("beta", [d, 1], F32, isOutput=False)
    out_t = nc.declare_dram_parameter("out", [n_nodes, d], F32, isOutput=True)

    xb_t = nc.dram_tensor("xb", [n_nodes, d], BF16)          # bf16 copy of x
    h1s_t = nc.dram_tensor("h1s", [nw, d, 128], F32)          # h1 pre-act, feat-major
    h1a_t = nc.dram_tensor("h1a", [n_nodes, d], BF16)         # h1 post BN+relu, node-major

    n_blocks = (nw + block - 1) // block

    # ---------------- build ----------------
    with tile.TileContext(nc) as tc:
        # ---- phase 0: bf16 copy of x (HBM->HBM cast via SBUF tiles) ----
        with tc.tile_pool(name="cast", bufs=3) as castp:
            step = 384  # rows per tile chunk of 128 partitions: use row tiles
            # process x in [128, 512] f32 tiles via rearrange over row blocks
            nrb = (n_nodes + 511) // 512
            for i in range(nrb):
                r0 = i * 512
                rows = min(512, n_nodes - r0)
                prow = (rows + 127) // 128  # full 128-row subblocks; rows%128==0 except tail
                ft = castp.tile([128, 4, d], F32, tag="cf", name=f"cf{i}")
                bt = castp.tile([128, 4, d], BF16, tag="cb", name=f"cb{i}")
                if rows == 512:
                    src = x_t[r0:r0 + 512, :].rearrange("(a p) d -> p a d", p=128)
                    dstv = xb_t[r0:r0 + 512, :].rearrange("(a p) d -> p a d", p=128)
                    nc.sync.dma_start(ft[:], src)
                    nc.vector.tensor_copy(bt[:], ft[:])
                    nc.sync.dma_start(dstv, bt[:])
                else:
                    # tail: handle 128-row pieces and final partial
                    done = 0
                    while done < rows:
                        pr = min(128, rows - done)
                        srcv = x_t[r0 + done:r0 + done + pr, :]
                        dstv = xb_t[r0 + done:r0 + done + pr, :]
                        nc.sync.dma_start(ft[:pr, 0, :], srcv)
                        nc.vector.tensor_copy(bt[:pr, 0, :], ft[:pr, 0, :])
                        nc.sync.dma_start(dstv, bt[:pr, 0, :])
                        done += pr

        # ---- constants ----
        with tc.tile_pool(name="const", bufs=1) as constpool:
            iota_sb = constpool.tile([128, 128], F32)
            ident_sb = constpool.tile([128, 128], F32)
            identb_sb = constpool.tile([128, 128], BF16)
            invd_sb = constpool.tile([128, nw], F32)
            ws0_sb = constpool.tile([128, 128], F32)
            wn0_sb = constpool.tile([128, 128], F32)
            wn1_sb = constpool.tile([128, 128], F32)
            wsb_sb = constpool.tile([128, 128], BF16)
            b0_sb = constpool.tile([128, 1], F32)
            b1_sb = constpool.tile([128, 1], F32)
            gam_sb = constpool.tile([128, 1], F32)
            bet_sb = constpool.tile([128, 1], F32)
            stats_sum = constpool.tile([128, nw], F32)
            stats_sq = constpool.tile([128, nw], F32)
            a_sb = constpool.tile([128, 1], F32)
            c_sb = constpool.tile([128, 1], F32)
            nc.sync.dma_start(iota_sb[:], iota_t[:])
            nc.sync.dma_start(ident_sb[:], ident_t[:])
            nc.sync.dma_start(identb_sb[:], identb_t[:])
            nc.sync.dma_start(invd_sb[:], invd_t[:])
            nc.sync.dma_start(ws0_sb[:], ws_t[0])
            nc.sync.dma_start(wn0_sb[:], wn_t[0])
            nc.sync.dma_start(wn1_sb[:], wn_t[1])
            nc.sync.dma_start(wsb_sb[:], wsb_t[:])
            nc.sync.dma_start(b0_sb[:], b_t[0])
            nc.sync.dma_start(b1_sb[:], b_t[1])
            nc.sync.dma_start(gam_sb[:], gam_t[:])
            nc.sync.dma_start(bet_sb[:], bet_t[:])

            # ---- phase A: agg0 + layer0 + BN stats ----
            with (
                tc.tile_pool(name="gath", bufs=3) as gathp,
                tc.tile_pool(name="idxp", bufs=3) as idxp,
                tc.tile_pool(name="dlp", bufs=3) as dlp,
                tc.tile_pool(name="sp", bufs=6) as sp,
                tc.tile_pool(name="aggp", bufs=3) as aggp,
                tc.tile_pool(name="xp", bufs=3) as xp,
                tc.tile_pool(name="h1p", bufs=3) as h1p,
                tc.tile_pool(name="wpsp", bufs=1, space="PSUM") as wpsp,
                tc.tile_pool(name="pstp", bufs=3, space="PSUM") as pstp,
                tc.tile_pool(name="php", bufs=2, space="PSUM") as php,
            ):
                def consumeA(w, aT):
                    n = wsz(w)
                    xw = xp.tile([128, 128], F32, tag="xw", name=f"xw{w}")
                    nc.sync.dma_start(xw[:n, :], x_t[w * 128:w * 128 + n, :])
                    ptx = pstp.tile([128, 128], F32, tag="pt", name=f"ptx{w}")
                    nc.tensor.transpose(ptx[:, :n], xw[:n, :], ident_sb[:n, :n])
                    xT = xp.tile([128, 128], F32, tag="xT", name=f"xT{w}")
                    nc.vector.tensor_copy(xT[:, :n], ptx[:, :n])
                    hp = php.tile([128, 128], F32, tag="hp", name=f"hp{w}")
                    nc.tensor.matmul(hp[:, :n], ws0_sb[:], xT[:, :n],
                                     start=True, stop=False)
                    nc.tensor.matmul(hp[:, :n], wn0_sb[:], aT[:, :n],
                                     start=False, stop=True)
                    h1 = h1p.tile([128, 128], F32, tag="h1", name=f"h1_{w}")
                    nc.vector.tensor_scalar(h1[:, :n], hp[:, :n], b0_sb[:],
                                            None, OP.add, OP.add,
                                            accum_out=stats_sum[:, w:w + 1])
                    sq = h1p.tile([128, 128], F32, tag="sq", name=f"sq{w}")
                    nc.scalar.activation(sq[:, :n], h1[:, :n], AF.Square,
                                         accum_out=stats_sq[:, w:w + 1])
                    nc.sync.dma_start(h1s_t[w][:, :n], h1[:, :n])

                run_agg(nc, tc, gathers, first_mm, last_mm, n_blocks, block,
                        nw, d, iota_sb, ident_sb, invd_sb,
                        gathp, idxp, dlp, sp, aggp, pstp, wpsp,
                        lambda g: xb_t[g * group:min((g + 1) * group, n_nodes), :],
                        idx_t, dl_t, consumeA, "A")

            # ---- BN stat finalize ----
            with tc.tile_pool(name="bnf", bufs=1) as bnf:
                sum_tot = bnf.tile([128, 1], F32)
                sq_tot = bnf.tile([128, 1], F32)
                nc.vector.reduce_sum(sum_tot[:], stats_sum[:], AX.X)
                nc.vector.reduce_sum(sq_tot[:], stats_sq[:], AX.X)
                mean = bnf.tile([128, 1], F32)
                msq = bnf.tile([128, 1], F32)
                nc.scalar.mul(mean[:], sum_tot[:], 1.0 / n_nodes)
                nc.scalar.mul(msq[:], sq_tot[:], 1.0 / n_nodes)
                m2 = bnf.tile([128, 1], F32)
                nc.vector.tensor_scalar(m2[:], mean[:], mean[:], None, OP.mult)
                var = bnf.tile([128, 1], F32)
                nc.vector.tensor_scalar(var[:], msq[:], m2[:], None, OP.subtract)
                vare = bnf.tile([128, 1], F32)
                nc.vector.tensor_scalar(vare[:], var[:], float(EPS), None, OP.add)
                std = bnf.tile([128, 1], F32)
                nc.scalar.activation(std[:], vare[:], AF.Sqrt, bias=0.0)
                rstd = bnf.tile([128, 1], F32)
                nc.vector.reciprocal(rstd[:], std[:])
                nc.vector.tensor_scalar(a_sb[:], gam_sb[:], rstd[:], None, OP.mult)
                ma = bnf.tile([128, 1], F32)
                nc.vector.tensor_scalar(ma[:], mean[:], a_sb[:], None, OP.mult)
                nc.vector.tensor_scalar(c_sb[:], bet_sb[:], ma[:], None, OP.subtract)

            # ---- phase B: BN apply + relu -> h1a (bf16 node-major) ----
            with (
                tc.tile_pool(name="pb", bufs=4) as pb,
                tc.tile_pool(name="pbps", bufs=2, space="PSUM") as pbps,
            ):
                for w in range(nw):
                    n = wsz(w)
                    ht = pb.tile([128, 128], F32, tag="ht", name=f"bht{w}")
                    nc.sync.dma_start(ht[:, :n], h1s_t[w][:, :n])
                    ab = pb.tile([128, 128], BF16, tag="ab", name=f"bab{w}")
                    nc.scalar.activation(ab[:, :n], ht[:, :n], AF.Relu,
                                         bias=c_sb[:], scale=a_sb[:])
                    pt = pbps.tile([128, 128], BF16, tag="bpt", name=f"bpt{w}")
                    nc.tensor.transpose(pt[:n, :], ab[:, :n], identb_sb[:])
                    hn = pb.tile([128, 128], BF16, tag="hn", name=f"bhn{w}")
                    nc.vector.tensor_copy(hn[:n, :], pt[:n, :])
                    nc.sync.dma_start(h1a_t[w * 128:w * 128 + n, :], hn[:n, :])

            # ---- phase C: agg1 + layer1 -> out ----
            with (
                tc.tile_pool(name="gathC", bufs=3) as gathp,
                tc.tile_pool(name="idxpC", bufs=3) as idxp,
                tc.tile_pool(name="dlpC", bufs=3) as dlp,
                tc.tile_pool(name="spC", bufs=6) as sp,
                tc.tile_pool(name="aggpC", bufs=3) as aggp,
                tc.tile_pool(name="xpC", bufs=3) as xp,
                tc.tile_pool(name="h2p", bufs=3) as h2p,
                tc.tile_pool(name="wpspC", bufs=1, space="PSUM") as wpsp,
                tc.tile_pool(name="pstpC", bufs=3, space="PSUM") as pstp,
                tc.tile_pool(name="phpC", bufs=2, space="PSUM") as php,
            ):
                def consumeC(w, aT):
                    n = wsz(w)
                    hw = xp.tile([128, 128], BF16, tag="hw", name=f"chw{w}")
                    nc.sync.dma_start(hw[:n, :], h1a_t[w * 128:w * 128 + n, :])
                    pth = pstp.tile([128, 128], BF16, tag="pt", name=f"cpt{w}")
                    nc.tensor.transpose(pth[:, :n], hw[:n, :], identb_sb[:n, :n])
                    hT = xp.tile([128, 128], BF16, tag="hT", name=f"chT{w}")
                    nc.vector.tensor_copy(hT[:, :n], pth[:, :n])
                    hp = php.tile([128, 128], F32, tag="hp2", name=f"chp{w}")
                    nc.tensor.matmul(hp[:, :n], wsb_sb[:], hT[:, :n],
                                     start=True, stop=False)
                    nc.tensor.matmul(hp[:, :n], wn1_sb[:], aT[:, :n],
                                     start=False, stop=True)
                    h2T = h2p.tile([128, 128], F32, tag="h2T", name=f"ch2T{w}")
                    nc.vector.tensor_scalar(h2T[:, :n], hp[:, :n], b1_sb[:],
                                            None, OP.add)
                    pto = pstp.tile([128, 128], F32, tag="pt", name=f"cpto{w}")
                    nc.tensor.transpose(pto[:n, :], h2T[:, :n], ident_sb[:])
                    h2n = h2p.tile([128, 128], F32, tag="h2n", name=f"ch2n{w}")
                    nc.vector.tensor_copy(h2n[:n, :], pto[:n, :])
                    nc.sync.dma_start(out_t[w * 128:w * 128 + n, :], h2n[:n, :])

                run_agg(nc, tc, gathers, first_mm, last_mm, n_blocks, block,
                        nw, d, iota_sb, ident_sb, invd_sb,
                        gathp, idxp, dlp, sp, aggp, pstp, wpsp,
                        lambda g: h1a_t[g * group:min((g + 1) * group, n_nodes), :],
                        idx_t, dl_t, consumeC, "C")

    nc.compile()
    return nc


def run_agg(nc, tc, gathers, first_mm, last_mm, n_blocks, block, nw, d,
            iota_sb, ident_sb, invd_sb, gathp, idxp, dlp, sp, aggp, pstp,
            wpsp, src_fn, idx_t, dl_t, consume, tag):
    """Emit the aggregation instruction stream for one layer."""
    # group gathers by block (chunks of one gather all lie in one block)
    by_block = [[] for _ in range(n_blocks)]
    for gi, ga in enumerate(gathers):
        bi = ga["chunks"][0][0] // block
        by_block[bi].append((gi, ga))

    # per-bank (block, half) first/last matmul ids: start=True zeroes the
    # whole 2KB PSUM zero-region (= bank), so only the first matmul into a
    # bank may set start; later windows' slices zero on first touch.
    bank_first = {}
    bank_last = {}
    for bi in range(n_blocks):
        wlo = bi * block
        for gi, ga in by_block[bi]:
            for ci, (wf, span) in enumerate(ga["chunks"]):
                for k in range(span):
                    key = (bi, (wf + k - wlo) // 4)
                    bank_first.setdefault(key, (gi, ci, k))
                    bank_last[key] = (gi, ci, k)

    seen = set()
    for bi in range(n_blocks):
        wlo = bi * block
        whi = min(wlo + block, nw)
        wtiles = {}

        def pslice(w):
            half = (w - wlo) // 4
            if half not in wtiles:
                wtiles[half] = wpsp.tile(
                    [128, 512], mybir.dt.float32, tag=f"wps{half}",
                    name=f"wps_{tag}_{bi}_{half}")
            off = ((w - wlo) % 4) * 128
            return wtiles[half][:, off:off + 128]

        def finish_window(w):
            aw = aggp.tile([128, 128], mybir.dt.float32, tag="agg",
                           name=f"agg_{tag}_{w}")
            nc.vector.tensor_scalar(aw[:], pslice(w), invd_sb[:, w:w + 1],
                                    None, OP.mult)
            pt = pstp.tile([128, 128], mybir.dt.float32, tag="pt",
                           name=f"pt_{tag}_{w}")
            nc.tensor.transpose(pt[:], aw[:], ident_sb[:])
            aT = aggp.tile([128, 128], mybir.dt.float32, tag="aggT",
                           name=f"aggT_{tag}_{w}")
            nc.vector.tensor_copy(aT[:], pt[:])
            consume(w, aT)

        for gi, ga in by_block[bi]:
            nidx = ga["nidx"]
            C = nidx // 128
            i16c = nidx // 16
            idx_sb = idxp.tile([128, i16c], I16, tag="idx",
                               name=f"idx_{tag}_{gi}")
            nc.sync.dma_start(
                idx_sb[:], idx_t[:, ga["idx_off"]:ga["idx_off"] + i16c])
            dl_sb = dlp.tile([128, C], F32, tag="dl", name=f"dl_{tag}_{gi}")
            nc.sync.dma_start(
                dl_sb[:], dl_t[:, ga["dl_off"]:ga["dl_off"] + C])
            gdst = gathp.tile([128, C, d], BF16, tag="gd",
                              name=f"gd_{tag}_{gi}")
            nc.gpsimd.dma_gather(gdst[:], src_fn(ga["g"]), idx_sb[:],
                                 nidx, nidx, d, single_packet=False,
                                 queue_num=gi % getattr(nc, "_kq", 1))
            for ci, (wf, span) in enumerate(ga["chunks"]):
                for k in range(span):
                    w = wf + k
                    S = sp.tile([128, 128], BF16, tag="S",
                                name=f"S_{tag}_{gi}_{ci}_{k}")
                    nc.vector.tensor_scalar(
                        S[:], iota_sb[:], dl_sb[:, ci:ci + 1],
                        float(-128 * k), OP.subtract, OP.is_equal)
                    key = (bi, (w - wlo) // 4)
                    is_bank_last = bank_last[key] == (gi, ci, k)
                    nc.tensor.matmul(
                        pslice(w), S[:], gdst[:, ci, :],
                        start=bank_first[key] == (gi, ci, k),
                        stop=is_bank_last)
                    seen.add(w)
                    if is_bank_last:
                        half = (w - wlo) // 4
                        for wv in range(wlo + half * 4,
                                        min(wlo + half * 4 + 4, whi)):
                            if wv in seen:
                                finish_window(wv)

        for w in range(wlo, whi):
            if w not in seen:
                seen.add(w)
                aT = aggp.tile([128, 128], mybir.dt.float32, tag="aggT",
                               name=f"aggzT_{tag}_{w}")
                nc.vector.memset(aT[:], 0.0)
                consume(w, aT)


# --------------------------------------------------------------------------
# public entry point
# --------------------------------------------------------------------------
def kernel(x, edge_index, W_self, W_nbr, b, gamma, beta):
    x = np.asarray(x, dtype=np.float32)
    edge_index = np.asarray(edge_index)
    W_self = np.asarray(W_self, dtype=np.float32)
    W_nbr = np.asarray(W_nbr, dtype=np.float32)
    b = np.asarray(b, dtype=np.float32)
    gamma = np.asarray(gamma, dtype=np.float32)
    beta = np.asarray(beta, dtype=np.float32)

    n_nodes, d = x.shape
    n_experts = W_self.shape[0]

    meta = preprocess(edge_index, n_nodes)
    nc = build_program(meta, n_nodes, d)

    iota_np = np.tile(np.arange(128, dtype=np.float32)[None, :], (128, 1))
    in_common = {
        "x": x,
        "idx": meta["idx_arr"],
        "dl": np.asarray(meta["dl_arr"]),
        "invd": meta["inv_col"],
        "iota": iota_np,
        "ident": np.eye(128, dtype=np.float32),
        "identb": np.eye(128, dtype=ml_dtypes.bfloat16),
    }
    in_maps = []
    for e in range(n_experts):
        m = dict(in_common)
        m["Wself"] = W_self[e]
        m["Wnbr"] = W_nbr[e]
        m["Wselfb"] = W_self[e, 1].astype(ml_dtypes.bfloat16)
        m["bias"] = b[e][:, :, None]
        m["gamma"] = gamma[e, 0][:, None]
        m["beta"] = beta[e, 0][:, None]
        in_maps.append(m)

    res = run_bass_kernel_spmd(nc, in_maps, list(range(n_experts)))
    outs = [np.asarray(res.results[e]["out"]) for e in range(n_experts)]
    return np.stack(outs, axis=-1)



# revision 4
# speedup vs baseline: 1.0523x; 1.0523x over previous
"""Trainium2 Bass kernel for nn_EuclideanExperts (8-expert 2-layer GraphSAGE).

Expert-parallel: each of the 8 NeuronCores runs one expert's full encoder.
The graph aggregation (mean over in-neighbors) is computed as a sequence of
one-hot matmuls: edges sorted by destination window are gathered 128 at a
time with dma_gather (bf16 rows), a one-hot selection matrix S is built on
the vector engine from destination offsets, and S.T @ G accumulates into a
PSUM tile holding the window's aggregate.  Dense layer matmuls, BatchNorm
and ReLU run in a feature-major layout (features on partitions) so BN
reductions are free-axis reductions and the BN+ReLU apply is a single
scalar-engine activation per window.

Self-contained: only numpy + the concourse stack from /opt/trn_rl_repo.
"""
import sys

for _p in ("/opt/trn_rl_repo", "/root/.axon_site/_ro/trn_rl_repo"):
    if _p not in sys.path:
        sys.path.insert(0, _p)

import os

import numpy as np
import ml_dtypes

import concourse.bacc as bacc
import concourse.mybir as mybir
import concourse.tile as tile
from concourse.bass_utils import run_bass_kernel_spmd

F32 = mybir.dt.float32
BF16 = mybir.dt.bfloat16
I16 = mybir.dt.int16
AX = mybir.AxisListType
OP = mybir.AluOpType
AF = mybir.ActivationFunctionType

EPS = 1e-5


# --------------------------------------------------------------------------
# host-side graph preprocessing (index data only; no float math on x)
# --------------------------------------------------------------------------
def preprocess(edge_index, n_nodes, block=8, group=25000, gmax=4096):
    """Sort edges into (window-block, src-group) runs and chunk them.

    Returns metadata driving the bass program plus the packed index arrays.
    A "window" is 128 consecutive destination nodes (one PSUM tile's rows).
    A "run" is all edges with dst in one block and src in one group, sorted
    by dst, split into gathers of <= gmax indices (multiple of 128).
    """
    src = np.asarray(edge_index[0], dtype=np.int64)
    dst = np.asarray(edge_index[1], dtype=np.int64)
    E = src.shape[0]
    nw = (n_nodes + 127) // 128
    n_groups = (n_nodes + group - 1) // group

    deg = np.bincount(dst, minlength=n_nodes).astype(np.float32)
    inv_deg = (1.0 / np.maximum(deg, 1.0)).astype(np.float32)
    inv_pad = np.ones(nw * 128, np.float32)
    inv_pad[:n_nodes] = inv_deg
    inv_col = inv_pad.reshape(nw, 128).T.copy()  # [128, nw]

    # order all edges by (block, group, dst) in a single argsort
    blk = dst // (128 * block)
    grp = src // group
    key = (blk * n_groups + grp) * np.int64(n_nodes) + dst
    order = np.argsort(key, kind="stable")
    s_s, s_d, s_b, s_g = src[order], dst[order], blk[order], grp[order]

    # run boundaries: change of (blk, grp)
    rk = s_b * n_groups + s_g
    bounds = np.flatnonzero(np.diff(rk)) + 1
    starts = np.concatenate([[0], bounds])
    ends = np.concatenate([bounds, [E]])

    gathers = []          # per gather: dict(g, idx_off, nidx, dl_off, chunks)
    idx_parts = []        # wrapped int16 [16, n/16] pieces (replicated later)
    dl_parts = []         # [128, C] f32 pieces
    idx_cursor = 0        # in int16 columns (16-wrapped)
    dl_cursor = 0         # in chunks
    for s, e in zip(starts, ends):
        g = int(s_g[s])
        for q in range(s, e, gmax):
            qe = min(q + gmax, e)
            es = s_s[q:qe] - g * group
            ed = s_d[q:qe]
            ne = qe - q
            npad = (-ne) % 128
            idxs = np.concatenate([es, np.zeros(npad, np.int64)])
            dabs = np.concatenate([ed, np.full(npad, -1, np.int64)])
            nidx = ne + npad
            C = nidx // 128
            chunks = []
            dl_rel = np.empty(nidx, np.float32)
            for c in range(C):
                dc = dabs[c * 128:(c + 1) * 128]
                valid = dc >= 0
                wfirst = int(dc[valid].min()) // 128
                wlast = int(dc[valid].max()) // 128
                dl_rel[c * 128:(c + 1) * 128] = np.where(
                    valid, dc - wfirst * 128, -1)
                chunks.append((wfirst, wlast - wfirst + 1))
            idx_parts.append(idxs.reshape(-1, 16).T.astype(np.int16))
            dl_parts.append(dl_rel.reshape(C, 128).T)
            gathers.append(dict(g=g, idx_off=idx_cursor, nidx=nidx,
                                dl_off=dl_cursor, chunks=chunks))
            idx_cursor += nidx // 16
            dl_cursor += C

    idx_arr = np.tile(np.concatenate(idx_parts, axis=1), (8, 1))  # [128, TI]
    dl_arr = np.concatenate(dl_parts, axis=1).astype(np.float32)

    # per-window first/last matmul id for start/stop flags
    first_mm = {}
    last_mm = {}
    for gi, ga in enumerate(gathers):
        for ci, (wf, span) in enumerate(ga["chunks"]):
            for k in range(span):
                w = wf + k
                first_mm.setdefault(w, (gi, ci, k))
                last_mm[w] = (gi, ci, k)

    return dict(gathers=gathers, idx_arr=idx_arr, dl_arr=dl_arr,
                inv_col=inv_col, first_mm=first_mm, last_mm=last_mm,
                nw=nw, n_groups=n_groups, block=block, group=group)


# --------------------------------------------------------------------------
# bass program
# --------------------------------------------------------------------------
def build_program(meta, n_nodes, d=128):
    nw = meta["nw"]
    block = meta["block"]
    group = meta["group"]
    gathers = meta["gathers"]
    first_mm = meta["first_mm"]
    last_mm = meta["last_mm"]
    TI = meta["idx_arr"].shape[1]
    TC = meta["dl_arr"].shape[1]

    def wsz(w):
        return min(128, n_nodes - w * 128)

    n_queues = int(os.environ.get("KERNEL_QUEUES", "4"))
    nc = bacc.Bacc("TRN2", target_bir_lowering=False, debug=False,
                   num_swdge_queues=n_queues)
    nc._kq = n_queues
    x_t = nc.declare_dram_parameter("x", [n_nodes, d], F32, isOutput=False)
    idx_t = nc.declare_dram_parameter("idx", [128, TI], I16, isOutput=False)
    dl_t = nc.declare_dram_parameter("dl", [128, TC], F32, isOutput=False)
    invd_t = nc.declare_dram_parameter("invd", [128, nw], F32, isOutput=False)
    iota_t = nc.declare_dram_parameter("iota", [128, 128], F32, isOutput=False)
    ident_t = nc.declare_dram_parameter("ident", [128, 128], F32, isOutput=False)
    identb_t = nc.declare_dram_parameter("identb", [128, 128], BF16, isOutput=False)
    ws_t = nc.declare_dram_parameter("Wself", [2, d, d], F32, isOutput=False)
    wn_t = nc.declare_dram_parameter("Wnbr", [2, d, d], F32, isOutput=False)
    wsb_t = nc.declare_dram_parameter("Wselfb", [d, d], BF16, isOutput=False)  # layer-1 self, bf16
    b_t = nc.declare_dram_parameter("bias", [2, d, 1], F32, isOutput=False)
    gam_t = nc.declare_dram_parameter("gamma", [d, 1], F32, isOutput=False)
    bet_t = nc.declare_dram_parameter("beta", [d, 1], F32, isOutput=False)
    out_t = nc.declare_dram_parameter("out", [n_nodes, d], F32, isOutput=True)

    xb_t = nc.dram_tensor("xb", [n_nodes, d], BF16)          # bf16 copy of x
    h1s_t = nc.dram_tensor("h1s", [nw, d, 128], F32)          # h1 pre-act, feat-major
    h1a_t = nc.dram_tensor("h1a", [n_nodes, d], BF16)         # h1 post BN+relu, node-major

    n_blocks = (nw + block - 1) // block

    # ---------------- build ----------------
    with tile.TileContext(nc) as tc:
        # ---- phase 0: bf16 copy of x (HBM->HBM cast via SBUF tiles) ----
        with tc.tile_pool(name="cast", bufs=3) as castp:
            step = 384  # rows per tile chunk of 128 partitions: use row tiles
            # process x in [128, 512] f32 tiles via rearrange over row blocks
            nrb = (n_nodes + 511) // 512
            for i in range(nrb):
                r0 = i * 512
                rows = min(512, n_nodes - r0)
                prow = (rows + 127) // 128  # full 128-row subblocks; rows%128==0 except tail
                ft = castp.tile([128, 4, d], F32, tag="cf", name=f"cf{i}")
                bt = castp.tile([128, 4, d], BF16, tag="cb", name=f"cb{i}")
                if rows == 512:
                    src = x_t[r0:r0 + 512, :].rearrange("(a p) d -> p a d", p=128)
                    dstv = xb_t[r0:r0 + 512, :].rearrange("(a p) d -> p a d", p=128)
                    nc.sync.dma_start(ft[:], src)
                    nc.vector.tensor_copy(bt[:], ft[:])
                    nc.sync.dma_start(dstv, bt[:])
                else:
                    # tail: handle 128-row pieces and final partial
                    done = 0
                    while done < rows:
                        pr = min(128, rows - done)
                        srcv = x_t[r0 + done:r0 + done + pr, :]
                        dstv = xb_t[r0 + done:r0 + done + pr, :]
                        nc.sync.dma_start(ft[:pr, 0, :], srcv)
                        nc.vector.tensor_copy(bt[:pr, 0, :], ft[:pr, 0, :])
                        nc.sync.dma_start(dstv, bt[:pr, 0, :])
                        done += pr

        # ---- constants ----
        with tc.tile_pool(name="const", bufs=1) as constpool:
            iota_sb = constpool.tile([128, 128], F32)
            ident_sb = constpool.tile([128, 128], F32)
            identb_sb = constpool.tile([128, 128], BF16)
            invd_sb = constpool.tile([128, nw], F32)
            ws0_sb = constpool.tile([128, 128], F32)
            wn0_sb = constpool.tile([128, 128], F32)
            wn1_sb = constpool.tile([128, 128], F32)
            wsb_sb = constpool.tile([128, 128], BF16)
            b0_sb = constpool.tile([128, 1], F32)
            b1_sb = constpool.tile([128, 1], F32)
            gam_sb = constpool.tile([128, 1], F32)
            bet_sb = constpool.tile([128, 1], F32)
            stats_sum = constpool.tile([128, nw], F32)
            stats_sq = constpool.tile([128, nw], F32)
            a_sb = constpool.tile([128, 1], F32)
            c_sb = constpool.tile([128, 1], F32)
            nc.sync.dma_start(iota_sb[:], iota_t[:])
            nc.sync.dma_start(ident_sb[:], ident_t[:])
            nc.sync.dma_start(identb_sb[:], identb_t[:])
            nc.sync.dma_start(invd_sb[:], invd_t[:])
            nc.sync.dma_start(ws0_sb[:], ws_t[0])
            nc.sync.dma_start(wn0_sb[:], wn_t[0])
            nc.sync.dma_start(wn1_sb[:], wn_t[1])
            nc.sync.dma_start(wsb_sb[:], wsb_t[:])
            nc.sync.dma_start(b0_sb[:], b_t[0])
            nc.sync.dma_start(b1_sb[:], b_t[1])
            nc.sync.dma_start(gam_sb[:], gam_t[:])
            nc.sync.dma_start(bet_sb[:], bet_t[:])

            # ---- phase A: agg0 + layer0 + BN stats ----
            with (
                tc.tile_pool(name="gath", bufs=6) as gathp,
                tc.tile_pool(name="idxp", bufs=6) as idxp,
                tc.tile_pool(name="dlp", bufs=6) as dlp,
                tc.tile_pool(name="sp", bufs=6) as sp,
                tc.tile_pool(name="aggp", bufs=3) as aggp,
                tc.tile_pool(name="xp", bufs=3) as xp,
                tc.tile_pool(name="h1p", bufs=3) as h1p,
                tc.tile_pool(name="wpsp", bufs=1, space="PSUM") as wpsp,
                tc.tile_pool(name="pstp", bufs=3, space="PSUM") as pstp,
                tc.tile_pool(name="php", bufs=2, space="PSUM") as php,
            ):
                def consumeA(w, aT):
                    n = wsz(w)
                    xw = xp.tile([128, 128], F32, tag="xw", name=f"xw{w}")
                    nc.sync.dma_start(xw[:n, :], x_t[w * 128:w * 128 + n, :])
                    ptx = pstp.tile([128, 128], F32, tag="pt", name=f"ptx{w}")
                    nc.tensor.transpose(ptx[:, :n], xw[:n, :], ident_sb[:n, :n])
                    xT = xp.tile([128, 128], F32, tag="xT", name=f"xT{w}")
                    nc.vector.tensor_copy(xT[:, :n], ptx[:, :n])
                    hp = php.tile([128, 128], F32, tag="hp", name=f"hp{w}")
                    nc.tensor.matmul(hp[:, :n], ws0_sb[:], xT[:, :n],
                                     start=True, stop=False)
                    nc.tensor.matmul(hp[:, :n], wn0_sb[:], aT[:, :n],
                                     start=False, stop=True)
                    h1 = h1p.tile([128, 128], F32, tag="h1", name=f"h1_{w}")
                    nc.vector.tensor_scalar(h1[:, :n], hp[:, :n], b0_sb[:],
                                            None, OP.add, OP.add,
                                            accum_out=stats_sum[:, w:w + 1])
                    sq = h1p.tile([128, 128], F32, tag="sq", name=f"sq{w}")
                    nc.scalar.activation(sq[:, :n], h1[:, :n], AF.Square,
                                         accum_out=stats_sq[:, w:w + 1])
                    nc.sync.dma_start(h1s_t[w][:, :n], h1[:, :n])

                run_agg(nc, tc, gathers, first_mm, last_mm, n_blocks, block,
                        nw, d, iota_sb, ident_sb, invd_sb,
                        gathp, idxp, dlp, sp, aggp, pstp, wpsp,
                        lambda g: xb_t[g * group:min((g + 1) * group, n_nodes), :],
                        idx_t, dl_t, consumeA, "A")

            # ---- BN stat finalize ----
            with tc.tile_pool(name="bnf", bufs=1) as bnf:
                sum_tot = bnf.tile([128, 1], F32)
                sq_tot = bnf.tile([128, 1], F32)
                nc.vector.reduce_sum(sum_tot[:], stats_sum[:], AX.X)
                nc.vector.reduce_sum(sq_tot[:], stats_sq[:], AX.X)
                mean = bnf.tile([128, 1], F32)
                msq = bnf.tile([128, 1], F32)
                nc.scalar.mul(mean[:], sum_tot[:], 1.0 / n_nodes)
                nc.scalar.mul(msq[:], sq_tot[:], 1.0 / n_nodes)
                m2 = bnf.tile([128, 1], F32)
                nc.vector.tensor_scalar(m2[:], mean[:], mean[:], None, OP.mult)
                var = bnf.tile([128, 1], F32)
                nc.vector.tensor_scalar(var[:], msq[:], m2[:], None, OP.subtract)
                vare = bnf.tile([128, 1], F32)
                nc.vector.tensor_scalar(vare[:], var[:], float(EPS), None, OP.add)
                std = bnf.tile([128, 1], F32)
                nc.scalar.activation(std[:], vare[:], AF.Sqrt, bias=0.0)
                rstd = bnf.tile([128, 1], F32)
                nc.vector.reciprocal(rstd[:], std[:])
                nc.vector.tensor_scalar(a_sb[:], gam_sb[:], rstd[:], None, OP.mult)
                ma = bnf.tile([128, 1], F32)
                nc.vector.tensor_scalar(ma[:], mean[:], a_sb[:], None, OP.mult)
                nc.vector.tensor_scalar(c_sb[:], bet_sb[:], ma[:], None, OP.subtract)

            # ---- phase B: BN apply + relu -> h1a (bf16 node-major) ----
            with (
                tc.tile_pool(name="pb", bufs=4) as pb,
                tc.tile_pool(name="pbps", bufs=2, space="PSUM") as pbps,
            ):
                for w in range(nw):
                    n = wsz(w)
                    ht = pb.tile([128, 128], F32, tag="ht", name=f"bht{w}")
                    nc.sync.dma_start(ht[:, :n], h1s_t[w][:, :n])
                    ab = pb.tile([128, 128], BF16, tag="ab", name=f"bab{w}")
                    nc.scalar.activation(ab[:, :n], ht[:, :n], AF.Relu,
                                         bias=c_sb[:], scale=a_sb[:])
                    pt = pbps.tile([128, 128], BF16, tag="bpt", name=f"bpt{w}")
                    nc.tensor.transpose(pt[:n, :], ab[:, :n], identb_sb[:])
                    hn = pb.tile([128, 128], BF16, tag="hn", name=f"bhn{w}")
                    nc.vector.tensor_copy(hn[:n, :], pt[:n, :])
                    nc.sync.dma_start(h1a_t[w * 128:w * 128 + n, :], hn[:n, :])

            # ---- phase C: agg1 + layer1 -> out ----
            with (
                tc.tile_pool(name="gathC", bufs=6) as gathp,
                tc.tile_pool(name="idxpC", bufs=6) as idxp,
                tc.tile_pool(name="dlpC", bufs=6) as dlp,
                tc.tile_pool(name="spC", bufs=6) as sp,
                tc.tile_pool(name="aggpC", bufs=3) as aggp,
                tc.tile_pool(name="xpC", bufs=3) as xp,
                tc.tile_pool(name="h2p", bufs=3) as h2p,
                tc.tile_pool(name="wpspC", bufs=1, space="PSUM") as wpsp,
                tc.tile_pool(name="pstpC", bufs=3, space="PSUM") as pstp,
                tc.tile_pool(name="phpC", bufs=2, space="PSUM") as php,
            ):
                def consumeC(w, aT):
                    n = wsz(w)
                    hw = xp.tile([128, 128], BF16, tag="hw", name=f"chw{w}")
                    nc.sync.dma_start(hw[:n, :], h1a_t[w * 128:w * 128 + n, :])
                    pth = pstp.tile([128, 128], BF16, tag="pt", name=f"cpt{w}")
                    nc.tensor.transpose(pth[:, :n], hw[:n, :], identb_sb[:n, :n])
                    hT = xp.tile([128, 128], BF16, tag="hT", name=f"chT{w}")
                    nc.vector.tensor_copy(hT[:, :n], pth[:, :n])
                    hp = php.tile([128, 128], F32, tag="hp2", name=f"chp{w}")
                    nc.tensor.matmul(hp[:, :n], wsb_sb[:], hT[:, :n],
                                     start=True, stop=False)
                    nc.tensor.matmul(hp[:, :n], wn1_sb[:], aT[:, :n],
                                     start=False, stop=True)
                    h2T = h2p.tile([128, 128], F32, tag="h2T", name=f"ch2T{w}")
                    nc.vector.tensor_scalar(h2T[:, :n], hp[:, :n], b1_sb[:],
                                            None, OP.add)
                    pto = pstp.tile([128, 128], F32, tag="pt", name=f"cpto{w}")
                    nc.tensor.transpose(pto[:n, :], h2T[:, :n], ident_sb[:])
                    h2n = h2p.tile([128, 128], F32, tag="h2n", name=f"ch2n{w}")
                    nc.vector.tensor_copy(h2n[:n, :], pto[:n, :])
                    nc.sync.dma_start(out_t[w * 128:w * 128 + n, :], h2n[:n, :])

                run_agg(nc, tc, gathers, first_mm, last_mm, n_blocks, block,
                        nw, d, iota_sb, ident_sb, invd_sb,
                        gathp, idxp, dlp, sp, aggp, pstp, wpsp,
                        lambda g: h1a_t[g * group:min((g + 1) * group, n_nodes), :],
                        idx_t, dl_t, consumeC, "C")

    nc.compile()
    return nc


def run_agg(nc, tc, gathers, first_mm, last_mm, n_blocks, block, nw, d,
            iota_sb, ident_sb, invd_sb, gathp, idxp, dlp, sp, aggp, pstp,
            wpsp, src_fn, idx_t, dl_t, consume, tag):
    """Emit the aggregation instruction stream for one layer."""
    # group gathers by block (chunks of one gather all lie in one block)
    by_block = [[] for _ in range(n_blocks)]
    for gi, ga in enumerate(gathers):
        bi = ga["chunks"][0][0] // block
        by_block[bi].append((gi, ga))

    # per-bank (block, half) first/last matmul ids: start=True zeroes the
    # whole 2KB PSUM zero-region (= bank), so only the first matmul into a
    # bank may set start; later windows' slices zero on first touch.
    bank_first = {}
    bank_last = {}
    for bi in range(n_blocks):
        wlo = bi * block
        for gi, ga in by_block[bi]:
            for ci, (wf, span) in enumerate(ga["chunks"]):
                for k in range(span):
                    key = (bi, (wf + k - wlo) // 4)
                    bank_first.setdefault(key, (gi, ci, k))
                    bank_last[key] = (gi, ci, k)

    seen = set()
    for bi in range(n_blocks):
        wlo = bi * block
        whi = min(wlo + block, nw)
        wtiles = {}

        def pslice(w):
            half = (w - wlo) // 4
            if half not in wtiles:
                wtiles[half] = wpsp.tile(
                    [128, 512], mybir.dt.float32, tag=f"wps{half}",
                    name=f"wps_{tag}_{bi}_{half}")
            off = ((w - wlo) % 4) * 128
            return wtiles[half][:, off:off + 128]

        def finish_window(w):
            aw = aggp.tile([128, 128], mybir.dt.float32, tag="agg",
                           name=f"agg_{tag}_{w}")
            nc.vector.tensor_scalar(aw[:], pslice(w), invd_sb[:, w:w + 1],
                                    None, OP.mult)
            pt = pstp.tile([128, 128], mybir.dt.float32, tag="pt",
                           name=f"pt_{tag}_{w}")
            nc.tensor.transpose(pt[:], aw[:], ident_sb[:])
            aT = aggp.tile([128, 128], mybir.dt.float32, tag="aggT",
                           name=f"aggT_{tag}_{w}")
            nc.vector.tensor_copy(aT[:], pt[:])
            consume(w, aT)

        for gi, ga in by_block[bi]:
            nidx = ga["nidx"]
            C = nidx // 128
            i16c = nidx // 16
            idx_sb = idxp.tile([128, i16c], I16, tag="idx",
                               name=f"idx_{tag}_{gi}")
            nc.sync.dma_start(
                idx_sb[:], idx_t[:, ga["idx_off"]:ga["idx_off"] + i16c])
            dl_sb = dlp.tile([128, C], F32, tag="dl", name=f"dl_{tag}_{gi}")
            nc.sync.dma_start(
                dl_sb[:], dl_t[:, ga["dl_off"]:ga["dl_off"] + C])
            gdst = gathp.tile([128, C, d], BF16, tag="gd",
                              name=f"gd_{tag}_{gi}")
            nc.gpsimd.dma_gather(gdst[:], src_fn(ga["g"]), idx_sb[:],
                                 nidx, nidx, d, single_packet=False,
                                 queue_num=gi % getattr(nc, "_kq", 1))
            for ci, (wf, span) in enumerate(ga["chunks"]):
                for k in range(span):
                    w = wf + k
                    S = sp.tile([128, 128], BF16, tag="S",
                                name=f"S_{tag}_{gi}_{ci}_{k}")
                    nc.vector.tensor_scalar(
                        S[:], iota_sb[:], dl_sb[:, ci:ci + 1],
                        float(-128 * k), OP.subtract, OP.is_equal)
                    key = (bi, (w - wlo) // 4)
                    is_bank_last = bank_last[key] == (gi, ci, k)
                    nc.tensor.matmul(
                        pslice(w), S[:], gdst[:, ci, :],
                        start=bank_first[key] == (gi, ci, k),
                        stop=is_bank_last)
                    seen.add(w)
                    if is_bank_last:
                        half = (w - wlo) // 4
                        for wv in range(wlo + half * 4,
                                        min(wlo + half * 4 + 4, whi)):
                            if wv in seen:
                                finish_window(wv)

        for w in range(wlo, whi):
            if w not in seen:
                seen.add(w)
                aT = aggp.tile([128, 128], mybir.dt.float32, tag="aggT",
                               name=f"aggzT_{tag}_{w}")
                nc.vector.memset(aT[:], 0.0)
                consume(w, aT)


# --------------------------------------------------------------------------
# public entry point
# --------------------------------------------------------------------------
def kernel(x, edge_index, W_self, W_nbr, b, gamma, beta):
    x = np.asarray(x, dtype=np.float32)
    edge_index = np.asarray(edge_index)
    W_self = np.asarray(W_self, dtype=np.float32)
    W_nbr = np.asarray(W_nbr, dtype=np.float32)
    b = np.asarray(b, dtype=np.float32)
    gamma = np.asarray(gamma, dtype=np.float32)
    beta = np.asarray(beta, dtype=np.float32)

    n_nodes, d = x.shape
    n_experts = W_self.shape[0]

    meta = preprocess(edge_index, n_nodes)
    nc = build_program(meta, n_nodes, d)

    iota_np = np.tile(np.arange(128, dtype=np.float32)[None, :], (128, 1))
    in_common = {
        "x": x,
        "idx": meta["idx_arr"],
        "dl": np.asarray(meta["dl_arr"]),
        "invd": meta["inv_col"],
        "iota": iota_np,
        "ident": np.eye(128, dtype=np.float32),
        "identb": np.eye(128, dtype=ml_dtypes.bfloat16),
    }
    in_maps = []
    for e in range(n_experts):
        m = dict(in_common)
        m["Wself"] = W_self[e]
        m["Wnbr"] = W_nbr[e]
        m["Wselfb"] = W_self[e, 1].astype(ml_dtypes.bfloat16)
        m["bias"] = b[e][:, :, None]
        m["gamma"] = gamma[e, 0][:, None]
        m["beta"] = beta[e, 0][:, None]
        in_maps.append(m)

    res = run_bass_kernel_spmd(nc, in_maps, list(range(n_experts)))
    outs = [np.asarray(res.results[e]["out"]) for e in range(n_experts)]
    return np.stack(outs, axis=-1)



# revision 17
# speedup vs baseline: 1.1386x; 1.0820x over previous
"""Trainium2 Bass kernel for nn_EuclideanExperts (8-expert 2-layer GraphSAGE).

Expert-parallel: each of the 8 NeuronCores runs one expert's full encoder.
The graph aggregation (mean over in-neighbors) is computed as a sequence of
one-hot matmuls: edges sorted by destination window are gathered 128 at a
time with dma_gather (bf16 rows), a one-hot selection matrix S is built on
the vector engine from destination offsets, and S.T @ G accumulates into a
PSUM tile holding the window's aggregate.  Dense layer matmuls, BatchNorm
and ReLU run in a feature-major layout (features on partitions) so BN
reductions are free-axis reductions and the BN+ReLU apply is a single
scalar-engine activation per window.

Throughput notes:
 - the dma_gather SWDGE ucode runs on one Q7 core pair per queue; issuing
   a block's gathers back-to-back on queues 0-3 engages all 8 Q7 cores.
 - the one-hot S tiles depend only on edge structure, which is shared by
   both layers: layer 0 builds them (one batched tile per gather) and
   stores to HBM; layer 1 reloads them with a single dense DMA per gather
   instead of rebuilding on the vector engine.

Self-contained: only numpy + the concourse stack from /opt/trn_rl_repo.
"""
import sys

for _p in ("/opt/trn_rl_repo", "/root/.axon_site/_ro/trn_rl_repo"):
    if _p not in sys.path:
        sys.path.insert(0, _p)

import os

import numpy as np
import ml_dtypes

import concourse.bacc as bacc
import concourse.mybir as mybir
import concourse.tile as tile
from concourse.bass_utils import run_bass_kernel_spmd

F32 = mybir.dt.float32
BF16 = mybir.dt.bfloat16
I16 = mybir.dt.int16
AX = mybir.AxisListType
OP = mybir.AluOpType
AF = mybir.ActivationFunctionType

EPS = 1e-5


# --------------------------------------------------------------------------
# host-side graph preprocessing (index data only; no float math on x)
# --------------------------------------------------------------------------
def preprocess(edge_index, n_nodes, block=8, group=25000, gmax=4096):
    """Sort edges into (window-block, src-group) runs and chunk them.

    Returns metadata driving the bass program plus the packed index arrays.
    A "window" is 128 consecutive destination nodes (one PSUM tile's rows).
    A "run" is all edges with dst in one block and src in one group, sorted
    by dst, split into gathers of <= gmax indices (multiple of 128).
    Each gather's matmul work is a list of "slots": one (chunk, window)
    pair per one-hot matrix.  dl holds one pre-biased offset column per
    slot so the S build is a single is_equal against iota.
    """
    src = np.asarray(edge_index[0], dtype=np.int64)
    dst = np.asarray(edge_index[1], dtype=np.int64)
    E = src.shape[0]
    nw = (n_nodes + 127) // 128
    n_groups = (n_nodes + group - 1) // group

    deg = np.bincount(dst, minlength=n_nodes).astype(np.float32)
    inv_deg = (1.0 / np.maximum(deg, 1.0)).astype(np.float32)
    inv_pad = np.ones(nw * 128, np.float32)
    inv_pad[:n_nodes] = inv_deg
    inv_col = inv_pad.reshape(nw, 128).T.copy()  # [128, nw]

    # order all edges by (block, group, dst) in a single argsort
    blk = dst // (128 * block)
    grp = src // group
    key = (blk * n_groups + grp) * np.int64(n_nodes) + dst
    order = np.argsort(key, kind="stable")
    s_s, s_d, s_b, s_g = src[order], dst[order], blk[order], grp[order]

    # run boundaries: change of (blk, grp)
    rk = s_b * n_groups + s_g
    bounds = np.flatnonzero(np.diff(rk)) + 1
    starts = np.concatenate([[0], bounds])
    ends = np.concatenate([bounds, [E]])

    gathers = []          # per gather: dict(g, idx_off, nidx, slot_off, slots)
    idx_parts = []        # wrapped int16 [16, n/16] pieces (replicated later)
    dl_parts = []         # [128, nslots] f32 pieces (pre-biased per slot)
    idx_cursor = 0        # in int16 columns (16-wrapped)
    slot_cursor = 0       # in slots
    for s, e in zip(starts, ends):
        g = int(s_g[s])
        for q in range(s, e, gmax):
            qe = min(q + gmax, e)
            es = s_s[q:qe] - g * group
            ed = s_d[q:qe]
            ne = qe - q
            npad = (-ne) % 128
            idxs = np.concatenate([es, np.zeros(npad, np.int64)])
            dabs = np.concatenate([ed, np.full(npad, -1, np.int64)])
            nidx = ne + npad
            C = nidx // 128
            slots = []    # (ci, w)
            dl_cols = []
            for c in range(C):
                dc = dabs[c * 128:(c + 1) * 128]
                valid = dc >= 0
                wfirst = int(dc[valid].min()) // 128
                wlast = int(dc[valid].max()) // 128
                rel = np.where(valid, dc - wfirst * 128, -1).astype(np.float32)
                for k in range(wlast - wfirst + 1):
                    slots.append((c, wfirst + k))
                    dl_cols.append(rel - 128.0 * k)
            idx_parts.append(idxs.reshape(-1, 16).T.astype(np.int16))
            dl_parts.append(np.stack(dl_cols, axis=1))
            gathers.append(dict(g=g, idx_off=idx_cursor, nidx=nidx,
                                slot_off=slot_cursor, slots=slots))
            idx_cursor += nidx // 16
            slot_cursor += len(slots)

    idx_arr = np.tile(np.concatenate(idx_parts, axis=1), (8, 1))  # [128, TI]
    dl_arr = np.concatenate(dl_parts, axis=1).astype(np.float32)

    return dict(gathers=gathers, idx_arr=idx_arr, dl_arr=dl_arr,
                inv_col=inv_col, nw=nw, n_groups=n_groups, block=block,
                group=group, n_slots=slot_cursor,
                max_slots=max(len(g["slots"]) for g in gathers))


# --------------------------------------------------------------------------
# bass program
# --------------------------------------------------------------------------
def build_program(meta, n_nodes, d=128):
    nw = meta["nw"]
    block = meta["block"]
    group = meta["group"]
    gathers = meta["gathers"]
    TI = meta["idx_arr"].shape[1]
    TS = meta["n_slots"]

    ms = meta["max_slots"]

    def wsz(w):
        return min(128, n_nodes - w * 128)

    n_queues = int(os.environ.get("KERNEL_QUEUES", "4"))
    nc = bacc.Bacc("TRN2", target_bir_lowering=False, debug=False,
                   num_swdge_queues=n_queues)
    nc._kq = n_queues
    x_t = nc.declare_dram_parameter("x", [n_nodes, d], F32, isOutput=False)
    idx_t = nc.declare_dram_parameter("idx", [128, TI], I16, isOutput=False)
    dl_t = nc.declare_dram_parameter("dl", [128, TS], F32, isOutput=False)
    invd_t = nc.declare_dram_parameter("invd", [128, nw], F32, isOutput=False)
    iota_t = nc.declare_dram_parameter("iota", [128, 128], F32, isOutput=False)
    ident_t = nc.declare_dram_parameter("ident", [128, 128], F32, isOutput=False)
    identb_t = nc.declare_dram_parameter("identb", [128, 128], BF16, isOutput=False)
    ws_t = nc.declare_dram_parameter("Wself", [2, d, d], F32, isOutput=False)
    wn_t = nc.declare_dram_parameter("Wnbr", [2, d, d], F32, isOutput=False)
    wsb_t = nc.declare_dram_parameter("Wselfb", [d, d], BF16, isOutput=False)  # layer-1 self, bf16
    b_t = nc.declare_dram_parameter("bias", [2, d, 1], F32, isOutput=False)
    gam_t = nc.declare_dram_parameter("gamma", [d, 1], F32, isOutput=False)
    bet_t = nc.declare_dram_parameter("beta", [d, 1], F32, isOutput=False)
    out_t = nc.declare_dram_parameter("out", [n_nodes, d], F32, isOutput=True)

    xb_t = nc.dram_tensor("xb", [n_nodes, d], BF16)          # bf16 copy of x
    h1s_t = nc.dram_tensor("h1s", [nw, d, 128], F32)          # h1 pre-act, feat-major
    h1a_t = nc.dram_tensor("h1a", [n_nodes, d], BF16)         # h1 post BN+relu, node-major
    # cached one-hot tiles, split into two tensors to stay under the 256MB
    # DRAM scratch page size; each gather's slot run lives in one tensor.
    half = TS // 2
    split_gi = next((i for i, g in enumerate(gathers)
                     if g["slot_off"] + len(g["slots"]) > half), len(gathers))
    split_slot = (gathers[split_gi]["slot_off"] if split_gi < len(gathers)
                  else TS)
    s_t0 = nc.dram_tensor("sone0", [128, max(split_slot, 1) * 128], BF16)
    s_t1 = nc.dram_tensor("sone1", [128, max(TS - split_slot, 1) * 128], BF16)

    def s_slice(slot_off, ns):
        if slot_off >= split_slot:
            off = slot_off - split_slot
            return s_t1[:, off * 128:(off + ns) * 128]
        return s_t0[:, slot_off * 128:(slot_off + ns) * 128]

    n_blocks = (nw + block - 1) // block

    # hoisted num_idxs registers (one per distinct gather length)
    nidx_regs = {}

    def nidx_reg(n):
        if n not in nidx_regs:
            nidx_regs[n] = nc.gpsimd.to_reg(n)
        return nidx_regs[n]

    # ---------------- build ----------------
    with tile.TileContext(nc) as tc:
        # ---- phase 0: bf16 copy of x (HBM->HBM cast via SBUF tiles) ----
        with tc.tile_pool(name="cast", bufs=3) as castp:
            nrb = (n_nodes + 511) // 512
            for i in range(nrb):
                r0 = i * 512
                rows = min(512, n_nodes - r0)
                ft = castp.tile([128, 4, d], F32, tag="cf", name=f"cf{i}")
                bt = castp.tile([128, 4, d], BF16, tag="cb", name=f"cb{i}")
                if rows == 512:
                    src = x_t[r0:r0 + 512, :].rearrange("(a p) d -> p a d", p=128)
                    dstv = xb_t[r0:r0 + 512, :].rearrange("(a p) d -> p a d", p=128)
                    nc.sync.dma_start(ft[:], src)
                    nc.vector.tensor_copy(bt[:], ft[:])
                    nc.sync.dma_start(dstv, bt[:])
                else:
                    done = 0
                    while done < rows:
                        pr = min(128, rows - done)
                        srcv = x_t[r0 + done:r0 + done + pr, :]
                        dstv = xb_t[r0 + done:r0 + done + pr, :]
                        nc.sync.dma_start(ft[:pr, 0, :], srcv)
                        nc.vector.tensor_copy(bt[:pr, 0, :], ft[:pr, 0, :])
                        nc.sync.dma_start(dstv, bt[:pr, 0, :])
                        done += pr

        # ---- constants ----
        with tc.tile_pool(name="const", bufs=1) as constpool:
            iota_sb = constpool.tile([128, 128], F32)
            ident_sb = constpool.tile([128, 128], F32)
            identb_sb = constpool.tile([128, 128], BF16)
            invd_sb = constpool.tile([128, nw], F32)
            ws0_sb = constpool.tile([128, 128], F32)
            wn0_sb = constpool.tile([128, 128], F32)
            wn1_sb = constpool.tile([128, 128], F32)
            wsb_sb = constpool.tile([128, 128], BF16)
            b0_sb = constpool.tile([128, 1], F32)
            b1_sb = constpool.tile([128, 1], F32)
            gam_sb = constpool.tile([128, 1], F32)
            bet_sb = constpool.tile([128, 1], F32)
            stats_sum = constpool.tile([128, nw], F32)
            stats_sq = constpool.tile([128, nw], F32)
            a_sb = constpool.tile([128, 1], F32)
            c_sb = constpool.tile([128, 1], F32)
            nc.sync.dma_start(iota_sb[:], iota_t[:])
            nc.sync.dma_start(ident_sb[:], ident_t[:])
            nc.sync.dma_start(identb_sb[:], identb_t[:])
            nc.sync.dma_start(invd_sb[:], invd_t[:])
            nc.sync.dma_start(ws0_sb[:], ws_t[0])
            nc.sync.dma_start(wn0_sb[:], wn_t[0])
            nc.sync.dma_start(wn1_sb[:], wn_t[1])
            nc.sync.dma_start(wsb_sb[:], wsb_t[:])
            nc.sync.dma_start(b0_sb[:], b_t[0])
            nc.sync.dma_start(b1_sb[:], b_t[1])
            nc.sync.dma_start(gam_sb[:], gam_t[:])
            nc.sync.dma_start(bet_sb[:], bet_t[:])

            # ---- phase A: agg0 + layer0 + BN stats ----
            with (
                tc.tile_pool(name="gath", bufs=8) as gathp,
                tc.tile_pool(name="idxp", bufs=8) as idxp,
                tc.tile_pool(name="dlp", bufs=8) as dlp,
                tc.tile_pool(name="sp", bufs=4) as sp,
                tc.tile_pool(name="aggp", bufs=3) as aggp,
                tc.tile_pool(name="xp", bufs=3) as xp,
                tc.tile_pool(name="h1p", bufs=3) as h1p,
                tc.tile_pool(name="wpsp", bufs=1, space="PSUM") as wpsp,
                tc.tile_pool(name="pstp", bufs=3, space="PSUM") as pstp,
                tc.tile_pool(name="php", bufs=2, space="PSUM") as php,
            ):
                def consumeA(w, aT):
                    n = wsz(w)
                    xw = xp.tile([128, 128], F32, tag="xw", name=f"xw{w}")
                    nc.sync.dma_start(xw[:n, :], x_t[w * 128:w * 128 + n, :])
                    ptx = pstp.tile([128, 128], F32, tag="pt", name=f"ptx{w}")
                    nc.tensor.transpose(ptx[:, :n], xw[:n, :], ident_sb[:n, :n])
                    xT = xp.tile([128, 128], F32, tag="xT", name=f"xT{w}")
                    nc.vector.tensor_copy(xT[:, :n], ptx[:, :n])
                    hp = php.tile([128, 128], F32, tag="hp", name=f"hp{w}")
                    nc.tensor.matmul(hp[:, :n], ws0_sb[:], xT[:, :n],
                                     start=True, stop=False)
                    nc.tensor.matmul(hp[:, :n], wn0_sb[:], aT[:, :n],
                                     start=False, stop=True)
                    h1 = h1p.tile([128, 128], F32, tag="h1", name=f"h1_{w}")
                    nc.vector.tensor_scalar(h1[:, :n], hp[:, :n], b0_sb[:],
                                            None, OP.add, OP.add,
                                            accum_out=stats_sum[:, w:w + 1])
                    sq = h1p.tile([128, 128], F32, tag="sq", name=f"sq{w}")
                    nc.scalar.activation(sq[:, :n], h1[:, :n], AF.Square,
                                         accum_out=stats_sq[:, w:w + 1])
                    nc.sync.dma_start(h1s_t[w][:, :n], h1[:, :n])

                run_agg(nc, tc, gathers, n_blocks, block, nw, d, ms,
                        iota_sb, ident_sb, invd_sb,
                        gathp, idxp, dlp, sp, aggp, pstp, wpsp,
                        lambda g: xb_t[g * group:min((g + 1) * group, n_nodes), :],
                        idx_t, dl_t, s_slice, consumeA, "A", nidx_reg,
                        build_s=True)

            # ---- BN stat finalize ----
            with tc.tile_pool(name="bnf", bufs=1) as bnf:
                sum_tot = bnf.tile([128, 1], F32)
                sq_tot = bnf.tile([128, 1], F32)
                nc.vector.reduce_sum(sum_tot[:], stats_sum[:], AX.X)
                nc.vector.reduce_sum(sq_tot[:], stats_sq[:], AX.X)
                mean = bnf.tile([128, 1], F32)
                msq = bnf.tile([128, 1], F32)
                nc.scalar.mul(mean[:], sum_tot[:], 1.0 / n_nodes)
                nc.scalar.mul(msq[:], sq_tot[:], 1.0 / n_nodes)
                m2 = bnf.tile([128, 1], F32)
                nc.vector.tensor_scalar(m2[:], mean[:], mean[:], None, OP.mult)
                var = bnf.tile([128, 1], F32)
                nc.vector.tensor_scalar(var[:], msq[:], m2[:], None, OP.subtract)
                vare = bnf.tile([128, 1], F32)
                nc.vector.tensor_scalar(vare[:], var[:], float(EPS), None, OP.add)
                std = bnf.tile([128, 1], F32)
                nc.scalar.activation(std[:], vare[:], AF.Sqrt, bias=0.0)
                rstd = bnf.tile([128, 1], F32)
                nc.vector.reciprocal(rstd[:], std[:])
                nc.vector.tensor_scalar(a_sb[:], gam_sb[:], rstd[:], None, OP.mult)
                ma = bnf.tile([128, 1], F32)
                nc.vector.tensor_scalar(ma[:], mean[:], a_sb[:], None, OP.mult)
                nc.vector.tensor_scalar(c_sb[:], bet_sb[:], ma[:], None, OP.subtract)

            # ---- phase B: BN apply + relu -> h1a (bf16 node-major) ----
            with (
                tc.tile_pool(name="pb", bufs=4) as pb,
                tc.tile_pool(name="pbps", bufs=2, space="PSUM") as pbps,
            ):
                for w in range(nw):
                    n = wsz(w)
                    ht = pb.tile([128, 128], F32, tag="ht", name=f"bht{w}")
                    nc.sync.dma_start(ht[:, :n], h1s_t[w][:, :n])
                    ab = pb.tile([128, 128], BF16, tag="ab", name=f"bab{w}")
                    nc.scalar.activation(ab[:, :n], ht[:, :n], AF.Relu,
                                         bias=c_sb[:], scale=a_sb[:])
                    pt = pbps.tile([128, 128], BF16, tag="bpt", name=f"bpt{w}")
                    nc.tensor.transpose(pt[:n, :], ab[:, :n], identb_sb[:])
                    hn = pb.tile([128, 128], BF16, tag="hn", name=f"bhn{w}")
                    nc.vector.tensor_copy(hn[:n, :], pt[:n, :])
                    nc.sync.dma_start(h1a_t[w * 128:w * 128 + n, :], hn[:n, :])

            # ---- phase C: agg1 + layer1 -> out ----
            with (
                tc.tile_pool(name="gathC", bufs=8) as gathp,
                tc.tile_pool(name="idxpC", bufs=8) as idxp,
                tc.tile_pool(name="dlpC", bufs=8) as dlp,
                tc.tile_pool(name="spC", bufs=4) as sp,
                tc.tile_pool(name="aggpC", bufs=3) as aggp,
                tc.tile_pool(name="xpC", bufs=3) as xp,
                tc.tile_pool(name="h2p", bufs=3) as h2p,
                tc.tile_pool(name="wpspC", bufs=1, space="PSUM") as wpsp,
                tc.tile_pool(name="pstpC", bufs=3, space="PSUM") as pstp,
                tc.tile_pool(name="phpC", bufs=2, space="PSUM") as php,
            ):
                def consumeC(w, aT):
                    n = wsz(w)
                    hw = xp.tile([128, 128], BF16, tag="hw", name=f"chw{w}")
                    nc.sync.dma_start(hw[:n, :], h1a_t[w * 128:w * 128 + n, :])
                    pth = pstp.tile([128, 128], BF16, tag="pt", name=f"cpt{w}")
                    nc.tensor.transpose(pth[:, :n], hw[:n, :], identb_sb[:n, :n])
                    hT = xp.tile([128, 128], BF16, tag="hT", name=f"chT{w}")
                    nc.vector.tensor_copy(hT[:, :n], pth[:, :n])
                    hp = php.tile([128, 128], F32, tag="hp2", name=f"chp{w}")
                    nc.tensor.matmul(hp[:, :n], wsb_sb[:], hT[:, :n],
                                     start=True, stop=False)
                    nc.tensor.matmul(hp[:, :n], wn1_sb[:], aT[:, :n],
                                     start=False, stop=True)
                    h2T = h2p.tile([128, 128], F32, tag="h2T", name=f"ch2T{w}")
                    nc.vector.tensor_scalar(h2T[:, :n], hp[:, :n], b1_sb[:],
                                            None, OP.add)
                    pto = pstp.tile([128, 128], F32, tag="pt", name=f"cpto{w}")
                    nc.tensor.transpose(pto[:n, :], h2T[:, :n], ident_sb[:])
                    h2n = h2p.tile([128, 128], F32, tag="h2n", name=f"ch2n{w}")
                    nc.vector.tensor_copy(h2n[:n, :], pto[:n, :])
                    nc.sync.dma_start(out_t[w * 128:w * 128 + n, :], h2n[:n, :])

                run_agg(nc, tc, gathers, n_blocks, block, nw, d, ms,
                        iota_sb, ident_sb, invd_sb,
                        gathp, idxp, dlp, sp, aggp, pstp, wpsp,
                        lambda g: h1a_t[g * group:min((g + 1) * group, n_nodes), :],
                        idx_t, dl_t, s_slice, consumeC, "C", nidx_reg,
                        build_s=False)

    nc.compile()
    return nc


def run_agg(nc, tc, gathers, n_blocks, block, nw, d, ms,
            iota_sb, ident_sb, invd_sb, gathp, idxp, dlp, sp, aggp, pstp,
            wpsp, src_fn, idx_t, dl_t, s_slice, consume, tag, nidx_reg,
            build_s):
    """Emit the aggregation instruction stream for one layer.

    build_s=True: build the one-hot tiles on the vector engine and store
    the whole batch to s_t.  build_s=False: reload them from s_t.
    """
    # group gathers by block (slots of one gather all lie in one block)
    by_block = [[] for _ in range(n_blocks)]
    for gi, ga in enumerate(gathers):
        bi = ga["slots"][0][1] // block
        by_block[bi].append((gi, ga))

    # per-bank (block, half) first/last matmul ids: start=True zeroes the
    # whole 2KB PSUM zero-region (= bank), so only the first matmul into a
    # bank may set start; later windows' slices zero on first touch.
    bank_first = {}
    bank_last = {}
    for bi in range(n_blocks):
        wlo = bi * block
        for gi, ga in by_block[bi]:
            for si, (ci, w) in enumerate(ga["slots"]):
                key = (bi, (w - wlo) // 4)
                bank_first.setdefault(key, (gi, si))
                bank_last[key] = (gi, si)

    seen = set()
    kq = getattr(nc, "_kq", 1)
    g_ctr = 0
    for bi in range(n_blocks):
        wlo = bi * block
        whi = min(wlo + block, nw)
        wtiles = {}

        def pslice(w):
            half = (w - wlo) // 4
            if half not in wtiles:
                wtiles[half] = wpsp.tile(
                    [128, 512], mybir.dt.float32, tag=f"wps{half}",
                    name=f"wps_{tag}_{bi}_{half}")
            off = ((w - wlo) % 4) * 128
            return wtiles[half][:, off:off + 128]

        def finish_window(w):
            aw = aggp.tile([128, 128], mybir.dt.float32, tag="agg",
                           name=f"agg_{tag}_{w}")
            nc.vector.tensor_scalar(aw[:], pslice(w), invd_sb[:, w:w + 1],
                                    None, OP.mult)
            pt = pstp.tile([128, 128], mybir.dt.float32, tag="pt",
                           name=f"pt_{tag}_{w}")
            nc.tensor.transpose(pt[:], aw[:], ident_sb[:])
            aT = aggp.tile([128, 128], mybir.dt.float32, tag="aggT",
                           name=f"aggT_{tag}_{w}")
            nc.vector.tensor_copy(aT[:], pt[:])
            consume(w, aT)

        blockg = by_block[bi]

        # 1) index loads + gathers for the whole block, rotating queues
        gtiles = {}
        stiles = {}
        for j, (gi, ga) in enumerate(blockg):
            nidx = ga["nidx"]
            C = nidx // 128
            i16c = nidx // 16
            ns = len(ga["slots"])
            idx_sb = idxp.tile([128, i16c], I16, tag="idx",
                               name=f"idx_{tag}_{gi}")
            nc.sync.dma_start(
                idx_sb[:], idx_t[:, ga["idx_off"]:ga["idx_off"] + i16c])
            if build_s:
                dl_sb = dlp.tile([128, ns], mybir.dt.float32, tag="dl",
                                 name=f"dl_{tag}_{gi}")
                nc.sync.dma_start(
                    dl_sb[:], dl_t[:, ga["slot_off"]:ga["slot_off"] + ns])
            gdst = gathp.tile([128, C, d], BF16, tag="gd",
                              name=f"gd_{tag}_{gi}")
            nc.gpsimd.dma_gather(gdst[:], src_fn(ga["g"]), idx_sb[:],
                                 nidx, nidx_reg(nidx), d, single_packet=False,
                                 queue_num=g_ctr % kq)
            g_ctr += 1
            gtiles[gi] = gdst
            Sg = sp.tile([128, ms, 128], BF16, tag="S",
                         name=f"S_{tag}_{gi}")
            stiles[gi] = Sg
            soff = ga["slot_off"]
            if build_s:
                for si in range(ns):
                    nc.vector.tensor_scalar(
                        Sg[:, si, :], iota_sb[:], dl_sb[:, si:si + 1],
                        None, OP.is_equal)
                nc.sync.dma_start(s_slice(soff, ns), Sg[:, :ns, :])
            else:
                nc.sync.dma_start(Sg[:, :ns, :], s_slice(soff, ns))

        # 2) matmuls + window finishes
        for gi, ga in blockg:
            gdst = gtiles[gi]
            Sg = stiles[gi]
            for si, (ci, w) in enumerate(ga["slots"]):
                key = (bi, (w - wlo) // 4)
                is_bank_last = bank_last[key] == (gi, si)
                nc.tensor.matmul(
                    pslice(w), Sg[:, si, :], gdst[:, ci, :],
                    start=bank_first[key] == (gi, si),
                    stop=is_bank_last)
                seen.add(w)
                if is_bank_last:
                    half = (w - wlo) // 4
                    for wv in range(wlo + half * 4,
                                    min(wlo + half * 4 + 4, whi)):
                        if wv in seen:
                            finish_window(wv)

        for w in range(wlo, whi):
            if w not in seen:
                seen.add(w)
                aT = aggp.tile([128, 128], mybir.dt.float32, tag="aggT",
                               name=f"aggzT_{tag}_{w}")
                nc.vector.memset(aT[:], 0.0)
                consume(w, aT)


# --------------------------------------------------------------------------
# public entry point
# --------------------------------------------------------------------------
def kernel(x, edge_index, W_self, W_nbr, b, gamma, beta):
    x = np.asarray(x, dtype=np.float32)
    edge_index = np.asarray(edge_index)
    W_self = np.asarray(W_self, dtype=np.float32)
    W_nbr = np.asarray(W_nbr, dtype=np.float32)
    b = np.asarray(b, dtype=np.float32)
    gamma = np.asarray(gamma, dtype=np.float32)
    beta = np.asarray(beta, dtype=np.float32)

    n_nodes, d = x.shape
    n_experts = W_self.shape[0]

    meta = preprocess(edge_index, n_nodes)
    nc = build_program(meta, n_nodes, d)

    iota_np = np.tile(np.arange(128, dtype=np.float32)[None, :], (128, 1))
    in_common = {
        "x": x,
        "idx": meta["idx_arr"],
        "dl": np.asarray(meta["dl_arr"]),
        "invd": meta["inv_col"],
        "iota": iota_np,
        "ident": np.eye(128, dtype=np.float32),
        "identb": np.eye(128, dtype=ml_dtypes.bfloat16),
    }
    in_maps = []
    for e in range(n_experts):
        m = dict(in_common)
        m["Wself"] = W_self[e]
        m["Wnbr"] = W_nbr[e]
        m["Wselfb"] = W_self[e, 1].astype(ml_dtypes.bfloat16)
        m["bias"] = b[e][:, :, None]
        m["gamma"] = gamma[e, 0][:, None]
        m["beta"] = beta[e, 0][:, None]
        in_maps.append(m)

    res = run_bass_kernel_spmd(nc, in_maps, list(range(n_experts)))
    outs = [np.asarray(res.results[e]["out"]) for e in range(n_experts)]
    return np.stack(outs, axis=-1)
